# revision 17
# baseline (speedup 1.0000x reference)
"""Trainium2 Bass kernel for KeyValueAttention (4-head masked attention, gated combine).

v5 strategy (8 NeuronCores, query-dim sharded, 512 queries/core):
  Transposed space throughout (keys/features on partitions, queries on free dim).
  Chunk-major schedule in TWO passes (heads {0,1} then {2,3}):
  - Both heads' scores for a chunk land in ONE [128, 1024] psum pair tile
    (2 banks), so each chunk needs ONE masked-exp op over 1024 columns
    (halves op count + cross-engine semaphores vs per-head ops).
  - psum banks: score pairs (3 rot x 2) + AV x2 = 8; V-build groups and the
    output projection ride the score-pair rotation.
  - Masked exp per chunk, two balanced paths:
      'C'  (DVE): custom fused op: cubic-poly exp(s/8) * fp8 mask (mask
           broadcast over the head dim of the pair tile).
      'Bp' (ACT+Pool): ACT pair exp then 2 gpsimd in-place mask multiplies.
  - em tiles hold a chunk PAIR x 2 heads [128, 2, 2, 512] fp8 so the AV
    DoubleRow matmul reads a strided [128, 2, 512] per head.
  - W2_h = Wk_h @ Wq_h^T is built on-device from weights only (before x_Q
    lands), so QW_h = W2_h @ x_Q^T needs just 2 chained matmuls per half:
    the first scores issue ~10us in instead of ~23us.
  - Epilogue (output projection, gated combine) dripped into the pass-B
    chunk stream; its accumulate runs on gpsimd from an ACT bf16 copy.

Host side only reshapes/slices/transposes/casts inputs (no reference math).
"""

import os
import numpy as np

NQ, NK, DC, A, H, DO = 4096, 8192, 256, 64, 4, 256
NCORES = 8
NQC = NQ // NCORES   # 512 queries per core
KC = 128             # keys per chunk
NKC = NK // KC       # 64 chunks
NPAIR = NKC // 2     # 32 chunk pairs

# Per-chunk exp path (both heads of the pass): 'C' = DVE fused,
# 'Bp' = ACT exp + gpsimd mask muls.  5 C : 3 Bp per 8 chunks.
CHUNK_TYPE = ["C", "Bp", "C", "C", "Bp", "C", "C", "Bp"]

_cache = {}


# ---------------------------------------------------------------------------
# exp polynomial fit (shared host/device constants)
# ---------------------------------------------------------------------------
def _fit_exp_poly(scale=0.125, lo=-0.85, hi=0.85):
    """p(x) = 1 + b1 x + b2 x^2 + b3 x^3 ~ exp(x*scale) for x*scale in [lo,hi],
    relative-error weighted, p(0)=1 pinned."""
    t = np.linspace(lo, hi, 40001)
    w = 1.0 / np.exp(t)
    Amat = np.stack([t, t * t, t ** 3], axis=1) * w[:, None]
    a = np.linalg.lstsq(Amat, (np.exp(t) - 1.0) * w, rcond=None)[0]
    return [float(a[0] * scale), float(a[1] * scale ** 2), float(a[2] * scale ** 3)]


POLY_B = _fit_exp_poly()


def _register_dve_exp_op():
    """Define + register the custom DVE op (idempotent)."""
    from concourse.dve_spec import Spec, Src0, Src1, C0, C1, C2, One, lower
    from concourse.dve_ops import (
        DveOp, OPS, CUSTOM_DVE_SPECS, _SUB_OPCODE_FOR_NAME, _CUSTOM_DVE_ROW_BASE,
    )
    from concourse.dve_table_gen import dve_ver_for
    from concourse.dve_uop import DveOpSpec

    name = "EXP_POLY_MASK_ANT"
    if name in _SUB_OPCODE_FOR_NAME:
        return next(op for op in OPS if op.name == name)

    body = (((Src0 * C2 + C1) * Src0 + C0) * Src0 + One) * Src1
    spec = Spec(
        body=body,
        reference=lambda in0, in1, s0, s1, imm2: (
            (((in0 * imm2 + s1) * in0 + s0) * in0 + 1.0) * in1
        ),
    )
    op = DveOp(name, spec, subdim=False, uops_sha={})
    ver = dve_ver_for("TRN2")
    op.uops_sha[ver] = DveOpSpec(
        name=name, opcode=31, uops=lower(spec, ver=ver), rd1_en=True
    ).sha(ver)
    OPS.append(op)
    CUSTOM_DVE_SPECS[name] = spec
    _SUB_OPCODE_FOR_NAME[name] = _CUSTOM_DVE_ROW_BASE + len(OPS) - 1
    return op


# ---------------------------------------------------------------------------
# kernel build
# ---------------------------------------------------------------------------
def _build_kernel():
    import concourse.bacc as bacc
    import concourse.mybir as mybir
    from concourse.tile import TileContext

    EXP_OP = _register_dve_exp_op()

    F32 = mybir.dt.float32
    BF16 = mybir.dt.bfloat16
    FP8 = mybir.dt.float8e4
    AF = mybir.ActivationFunctionType
    ALU = mybir.AluOpType
    DR = mybir.MatmulPerfMode.DoubleRow

    nc = bacc.Bacc(None, target_bir_lowering=False, debug=False)

    # ---- DRAM inputs (per core) ----
    xqtb = nc.dram_tensor("xqtb", [128, 2, NQC], BF16, kind="ExternalInput")
    xkt8 = nc.dram_tensor("xkt8", [128, 2, NK], FP8, kind="ExternalInput")
    wqTb = nc.dram_tensor("wqTb", [64, 2, H, 128], BF16, kind="ExternalInput")
    wkTb = nc.dram_tensor("wkTb", [64, 2, H, 128], BF16, kind="ExternalInput")
    wv8 = nc.dram_tensor("wv8", [128, 2, H * A], FP8, kind="ExternalInput")
    wgtb = nc.dram_tensor("wgtb", [128, 2, H], BF16, kind="ExternalInput")
    bg = nc.dram_tensor("bg", [H, 1], F32, kind="ExternalInput")
    wo = nc.dram_tensor("wo", [A, DO], F32, kind="ExternalInput")
    bo = nc.dram_tensor("bo", [1, DO], F32, kind="ExternalInput")
    i4 = nc.dram_tensor("i4", [H, H], F32, kind="ExternalInput")
    # mask duplicated along queries so a chunk's [128, 1024] slice covers
    # both heads of the score pair tile with a single-free-dim AP
    maskx = nc.dram_tensor("maskx", [NKC, 128, 2 * NQC], FP8,
                           kind="ExternalInput")
    out = nc.dram_tensor("out", [NQC, DO], F32, kind="ExternalOutput")

    with TileContext(nc) as tc:
        with tc.sbuf_pool(name="consts", bufs=1) as cpool:
            # ---- constant tiles + DMAs, ordered for earliest compute start --
            # scalar (ACT hwdge) queue: x_Q first (gates the QW build), then
            # the key-side tensors.
            xqtb_t = cpool.tile([128, 2, NQC], BF16)
            nc.scalar.dma_start(xqtb_t, xqtb[:])
            xkt_t = cpool.tile([128, 2, NK], FP8)
            nc.scalar.dma_start(xkt_t[:, :, 0:2048], xkt8[:, :, 0:2048])
            wv_t = cpool.tile([128, 2, H * A], FP8)
            nc.scalar.dma_start(wv_t, wv8[:])
            nc.scalar.dma_start(xkt_t[:, :, 2048:NK], xkt8[:, :, 2048:NK])
            wgt_t = cpool.tile([128, 2, H], BF16)
            nc.scalar.dma_start(wgt_t, wgtb[:])
            bg_t = cpool.tile([H, 1], F32)
            nc.scalar.dma_start(bg_t, bg[:])
            wo_t = cpool.tile([A, DO], F32)
            nc.scalar.dma_start(wo_t, wo[:])
            bo_t = cpool.tile([1, DO], F32)
            nc.scalar.dma_start(bo_t, bo[:])
            i4_t = cpool.tile([H, H], F32)
            nc.scalar.dma_start(i4_t, i4[:])
            # sync queue: W2 weight operands first, then the mask image.
            wqT_t = cpool.tile([64, 2, H, 128], BF16)
            nc.sync.dma_start(wqT_t, wqTb[:])
            wkT_t = cpool.tile([64, 2, H, 128], BF16)
            nc.sync.dma_start(wkT_t, wkTb[:])
            mask_sb = cpool.tile([128, NKC, 2 * NQC], FP8)
            MBATCH = [(0, 4), (4, 12), (12, 20), (20, 28), (28, 36), (36, 44),
                      (44, 52), (52, 64)]
            for lo, hi in MBATCH:
                nc.sync.dma_start(
                    mask_sb[:, lo:hi, :],
                    maskx[lo:hi].rearrange("c p q -> p c q"),
                )

            woaug = cpool.tile([A + 1, DO + 1], BF16)
            nc.vector.memset(woaug, 0.0)
            nc.vector.tensor_copy(woaug[:A, :DO], wo_t)
            nc.vector.memset(woaug[A : A + 1, DO : DO + 1], 1.0)
            ones1 = cpool.tile([1, 128], F32)
            nc.vector.memset(ones1, 1.0)

            # ---- persistent operand tiles ----
            # W2_h = Wk_h @ Wq_h^T in bf16, layout [c' pair, i, h, c]
            w2sb = cpool.tile([128, 2, H, DC], BF16)
            qw8 = [cpool.tile([128, 2, NQC], FP8, name=f"qw{h}") for h in range(H)]
            # last dim padded to 80 so the AV DoubleRow k-tile step is %16==0
            vaug = cpool.tile([128, H, NKC, 80], FP8)
            nc.gpsimd.memset(vaug[:, :, :, A : A + 1], 1.0)
            gates = cpool.tile([H, NQC], F32)
            gt_sb = cpool.tile([128, 4 * H], F32)
            boB_sb = cpool.tile([128, DO], F32)
            nh = [cpool.tile([A + 1, NQC], BF16, name=f"nh{h}") for h in range(H)]
            acc_a = [cpool.tile([128, DO], F32, name=f"acca{q}") for q in range(4)]
            acc_b = [cpool.tile([128, DO], F32, name=f"accb{q}") for q in range(4)]

            with (
                tc.psum_pool(name="pm", bufs=1) as pm,
                tc.sbuf_pool(name="ms", bufs=1) as ms,
            ):
                def sp_tile():
                    return pm.tile([128, 2 * NQC], F32, tag="sp", bufs=3,
                                   name="sp")

                def build_w2(h):
                    # W2_h[c', c] = sum_a Wq[c', a] Wk[c, a]
                    for i in range(2):
                        ps = sp_tile()
                        nc.tensor.matmul(
                            ps[:, :DC].rearrange("p (j m) -> p j m", j=2),
                            wqT_t[:, i, h, :],
                            wkT_t[:, :, h, :],
                            start=True, stop=True,
                        )
                        nc.vector.tensor_copy(w2sb[:, i, h, :], ps[:, :DC])

                def build_qw(h):
                    # QW_h = W2_h @ x_Q^T -> fp8 [128, 2, NQC] (c = half*128+p)
                    for half in range(2):
                        ps = sp_tile()
                        for i in range(2):
                            nc.tensor.matmul(
                                ps[:, :NQC],
                                w2sb[:, i, h, half * 128 : (half + 1) * 128],
                                xqtb_t[:, i, :],
                                start=(i == 0), stop=(i == 1),
                            )
                        if half == 0:
                            nc.scalar.copy(qw8[h][:, half, :], ps[:, :NQC])
                        else:
                            nc.vector.tensor_copy(qw8[h][:, half, :], ps[:, :NQC])

                def build_gates():
                    g_ps = sp_tile()
                    for i in range(2):
                        nc.tensor.matmul(
                            g_ps[0:4, :NQC], wgt_t[:, i, :], xqtb_t[:, i, :],
                            start=(i == 0), stop=(i == 1),
                        )
                    nc.scalar.activation(gates, g_ps[0:4, :NQC], AF.Sigmoid,
                                         bias=bg_t[:], scale=1.0)

                def build_gt():
                    gt_ps = sp_tile()
                    for q in range(4):
                        nc.tensor.transpose(
                            gt_ps[:, q * H : q * H + H],
                            gates[:, q * 128 : (q + 1) * 128],
                            i4_t[:],
                        )
                    nc.vector.tensor_copy(gt_sb, gt_ps[:, : 4 * H])

                def build_boB():
                    boB_ps = sp_tile()
                    nc.tensor.matmul(boB_ps[:, :DO], ones1, bo_t,
                                     start=True, stop=True)
                    nc.vector.tensor_copy(boB_sb, boB_ps[:, :DO])

                def vbuild_group(t):
                    # V for chunk-iters [t, t+4): pass = t//64, chunks t%64..
                    P, c0 = t // 64, t % 64
                    vg = sp_tile()
                    for s in range(4):
                        c = c0 + s
                        nc.tensor.matmul(
                            vg[:, s * 2 * A : (s + 1) * 2 * A],
                            xkt_t[:, :, c * KC : (c + 1) * KC],
                            wv_t[:, :, 2 * P * A : (2 * P + 2) * A],
                            start=True, stop=True, perf_mode=DR,
                        )
                    eng = (nc.scalar, nc.vector)[(t // 4) % 2]
                    src = vg[:, : 4 * 2 * A].rearrange("p (s h a) -> p s h a",
                                                       s=4, h=2)
                    dst = vaug[:, 2 * P : 2 * P + 2, c0 : c0 + 4, 0:A]
                    if eng is nc.scalar:
                        nc.scalar.copy(dst.rearrange("p h s a -> p s h a"), src)
                    else:
                        eng.tensor_copy(dst.rearrange("p h s a -> p s h a"), src)

                def epilogue_head(h, q):
                    # output projection + gated accumulate for head h, qtile q
                    p_ps = sp_tile()
                    nc.tensor.matmul(
                        p_ps[:, : DO + 1],
                        nh[h][:, q * 128 : (q + 1) * 128],
                        woaug,
                        start=True, stop=True,
                    )
                    rden = ms.tile([128, 1], F32, tag="rden", bufs=2,
                                   name="rden")
                    nc.vector.reciprocal(rden, p_ps[:, DO : DO + 1])
                    sc = ms.tile([128, 1], F32, tag="sc", bufs=2, name="sc")
                    nc.vector.tensor_mul(
                        sc, rden, gt_sb[:, q * H + h : q * H + h + 1]
                    )
                    prev = boB_sb if h == 0 else (acc_a[q] if h % 2 == 1 else acc_b[q])
                    dst = acc_a[q] if h % 2 == 0 else acc_b[q]
                    nc.vector.scalar_tensor_tensor(
                        dst, p_ps[:, :DO], sc, prev,
                        op0=ALU.mult, op1=ALU.add,
                    )
                    if h == H - 1:
                        nc.sync.dma_start(
                            out[q * 128 : (q + 1) * 128, :], dst
                        )

                # ---- prelude: W2 (weights only), QW h0/h1, first V groups --
                build_w2(0)
                build_qw(0)
                build_w2(1)
                build_qw(1)
                vbuild_group(0)
                vbuild_group(4)

                # deferred work dripped into the chunk stream: (iter, fn)
                drip = {
                    3: lambda: build_w2(2),
                    7: lambda: build_qw(2),
                    11: lambda: build_w2(3),
                    15: lambda: build_qw(3),
                    19: build_gates,
                    23: build_gt,
                    27: build_boB,
                }
                # head 0/1 epilogue dripped into pass B
                for i, (h, q) in enumerate([(hh, qq) for hh in (0, 1)
                                            for qq in range(4)]):
                    drip[64 + 5 + 7 * i] = (
                        lambda h=h, q=q: epilogue_head(h, q)
                    )

                # ---- two passes over keys, chunk-major, 2 heads each ----
                for P in range(2):
                    ha, hb = 2 * P, 2 * P + 1
                    av = [
                        pm.tile([A + 1, NQC], F32, tag=f"av{hi}", bufs=1,
                                name=f"av{hi}")
                        for hi in range(2)
                    ]
                    pend = []

                    def emit_av(pair, em):
                        for hi in range(2):
                            nc.tensor.matmul(
                                av[hi],
                                vaug[:, 2 * P + hi, 2 * pair : 2 * pair + 2,
                                     0 : A + 1],
                                em[:, :, hi, :],
                                start=(pair == 0), stop=(pair == NPAIR - 1),
                                perf_mode=DR,
                            )

                    em_cur = None
                    for c in range(NKC):
                        t = P * 64 + c
                        pair, slot = divmod(c, 2)
                        if t % 4 == 0 and t + 8 < 128:
                            vbuild_group(t + 8)
                        if t in drip:
                            drip[t]()

                        sp = sp_tile()
                        # the two score matmuls share the keys stationary
                        for hi, h in enumerate((ha, hb)):
                            nc.tensor.matmul(
                                sp[:, hi * NQC : (hi + 1) * NQC],
                                xkt_t[:, :, c * KC : (c + 1) * KC],
                                qw8[h],
                                start=True, stop=True,
                                perf_mode=DR,
                            )
                        # masked exp: one op for both heads of the chunk
                        if slot == 0:
                            em_cur = ms.tile([128, 2, 2, NQC], FP8, tag="em",
                                             bufs=3, name="em")
                        dst = em_cur[:, slot].rearrange("p h q -> p (h q)")
                        if CHUNK_TYPE[c % 8] == "C":
                            nc.vector._custom_dve(
                                EXP_OP, out=dst, in0=sp,
                                in1=mask_sb[:, c, :],
                                s0=POLY_B[0], s1=POLY_B[1], imm2=POLY_B[2],
                            )
                        else:  # Bp
                            nc.scalar.activation(
                                dst, sp, AF.Exp, bias=0.0, scale=0.125
                            )
                            for hi in range(2):
                                nc.gpsimd.tensor_mul(
                                    em_cur[:, slot, hi], em_cur[:, slot, hi],
                                    mask_sb[:, c, hi * NQC : (hi + 1) * NQC],
                                )
                        if slot == 1:
                            pend.append((pair, em_cur))
                            if len(pend) > 1:
                                emit_av(*pend.pop(0))
                    for item in pend:
                        emit_av(*item)

                    # numerator/denominator psum -> sbuf, frees AV banks
                    nc.scalar.copy(nh[ha], av[0])
                    nc.vector.tensor_copy(nh[hb], av[1])

                # ---- tail: heads 2/3 output projection ----
                for h in (2, 3):
                    for q in range(4):
                        epilogue_head(h, q)
    nc.finalize()
    return nc


# ---------------------------------------------------------------------------
# host-side input prep
# ---------------------------------------------------------------------------
def _to_f8(x):
    import ml_dtypes
    return np.ascontiguousarray(np.asarray(x, dtype=np.float32).astype(
        ml_dtypes.float8_e4m3fn))


def _to_bf16(x):
    import ml_dtypes
    return np.ascontiguousarray(np.asarray(x, dtype=np.float32).astype(
        ml_dtypes.bfloat16))


def _dr_c_layout(xT):
    """[C=256, N] -> [128, 2, N] with c = i*128 + p."""
    return np.ascontiguousarray(xT.reshape(2, 128, -1).transpose(1, 0, 2))


def _prep_shared(x_K, Wq, Wk, Wv, Wg, bg, Wo, bo):
    xkt = x_K.T  # [256, NK]
    xkt8 = _to_f8(_dr_c_layout(xkt))

    # wqTb[a, half, h, m] = Wq[h, 128*half + m, a]  (same layout as wkTb)
    wqTb = _to_bf16(Wq.reshape(H, 2, 128, A).transpose(3, 1, 0, 2))
    wkTb = _to_bf16(Wk.reshape(H, 2, 128, A).transpose(3, 1, 0, 2))
    arr = np.empty((128, 2, H * A), np.float32)
    for h in range(H):
        arr[:, :, h * A:(h + 1) * A] = Wv[h].reshape(2, 128, A).transpose(1, 0, 2)
    wv8 = _to_f8(arr)
    wgtb = _to_bf16(Wg.T.reshape(2, 128, H).transpose(1, 0, 2))
    return {
        "xkt8": xkt8, "wqTb": wqTb, "wkTb": wkTb, "wv8": wv8, "wgtb": wgtb,
        "bg": np.asarray(bg, np.float32).reshape(H, 1),
        "wo": np.ascontiguousarray(np.asarray(Wo, np.float32)),
        "bo": np.asarray(bo, np.float32).reshape(1, DO),
        "i4": np.eye(H, dtype=np.float32),
    }


def _prep_mask_core(mask_sl):
    """mask_sl: [NQC, NK] int32 -> maskx [NKC, 128, 2*NQC] fp8 0/1,
    duplicated along queries (covers both heads of a score pair tile)."""
    import ml_dtypes
    mt = mask_sl.T.astype(np.float32)  # [NK, NQC]
    m3 = mt.reshape(NKC, KC, NQC)
    m6 = np.concatenate([m3, m3], axis=2)
    return np.ascontiguousarray(m6.astype(ml_dtypes.float8_e4m3fn))


def kernel(x_Q, x_K, mask, Wq, Wk, Wv, Wg, bg, Wo, bo):
    from concourse.bass_utils import run_bass_kernel_spmd

    x_Q = np.asarray(x_Q, dtype=np.float32)
    x_K = np.asarray(x_K, dtype=np.float32)
    mask = np.asarray(mask, dtype=np.int32)

    shared = _prep_shared(
        x_K, np.asarray(Wq, np.float32), np.asarray(Wk, np.float32),
        np.asarray(Wv, np.float32), np.asarray(Wg, np.float32),
        bg, Wo, bo,
    )

    in_maps = []
    for cidx in range(NCORES):
        sl = slice(cidx * NQC, (cidx + 1) * NQC)
        xqt = x_Q[sl].T  # [256, NQC]
        m = {
            "xqtb": _to_bf16(_dr_c_layout(xqt)),
            "maskx": _prep_mask_core(mask[sl]),
        }
        m.update(shared)
        in_maps.append(m)

    if "nc" not in _cache:
        _cache["nc"] = _build_kernel()
    res = run_bass_kernel_spmd(
        _cache["nc"], in_maps, list(range(NCORES)),
        trace=bool(int(os.environ.get("BASS_KERNEL_TRACE", "0"))),
    )
    if res.exec_time_ns is not None:
        print(f"HW exec time: {res.exec_time_ns} ns")
    return np.concatenate([r["out"] for r in res.results], axis=0)
